# revision 12
# baseline (speedup 1.0000x reference)
"""Distributed Bass kernel for nn_Attention_75514114998541.

GQA attention block (16 Q heads / 4 KV heads, head_dim 128, hidden 2048,
B=2, S=2048) with per-head RMSNorm on q/k, causal softmax, output proj.

Sharding: 8 cores = 2 (batch) x 4 (head groups). Core 4*b+g handles batch b
and heads [4g, 4g+4) (= kv head g). Wq/Wk/Wv column-sharded, Wo row-sharded;
each core emits a partial [S, HID] output (bf16), host sums the 4 partials
per batch in fp32.

v2 design (vs fp32r baseline):
  * all SBUF activations/weights in bf16 (PSUM accumulation stays fp32):
    halves DMA, enables DVE 2x/4x perf modes. PE rate is the same as fp32r
    but per-instruction overheads shrink and diagonal matmuls can shrink
    below 256 columns.
  * single activation table: scalar engine only ever runs Copy / Square /
    Ln / Exp (all in natural_log_exp_and_others) -- the RMS-norm rsqrt is
    computed as exp(-0.5*ln(ms+eps)), so no 1283ns table reloads.
  * V is produced directly in [tok, d] layout (64 N=128 matmuls per strip
    instead of 16 N=512 + 4 PE transposes).
  * rms rows for 4 q heads + k are accumulated into one [5,512] PSUM tile
    (matmul writes at partition offset h) -> one Ln + one Exp for all 5.
  * softmax accumulator (denominator) adds run on DVE in bf16.
  * out-proj of strip s-1 is interleaved INSIDE strip s's attention k-loop
    (1 chunk per few tiles) to keep the PE p-state ramped; projections of
    strip s+1 are emitted BEFORE attention of strip s so the PE never
    waits on the scalar engine at strip boundaries.
Layouts: xT[hid, tok] (host pre-transpose) -> QT/KT[d, tok] -> ST[k, q]
  -> PT[k, q] -> OT[d, q] -> out[tok, hid].
"""
import contextlib
import ctypes
import os
import sys
import types

import numpy as np
import ml_dtypes

sys.path.insert(0, "/opt/trn_rl_repo")

import concourse.bacc as bacc
import concourse.mybir as mybir
import concourse.tile as tile
from concourse.bass_utils import run_bass_kernel_spmd

F32 = mybir.dt.float32
BF16 = mybir.dt.bfloat16

NCORES = 8
S = 2048            # sequence length (= tokens per batch)
HID = 2048          # hidden dim
D = 128             # head dim
HQ = 4              # q heads per core
STRIP = 512         # token strip (matmul moving free dim)
NSTRIP = S // STRIP          # 4
KT = HID // 128              # 16 hidden k-tiles
EPS = 1e-6
TRACE = os.environ.get("BASS_KERNEL_TRACE", "0") == "1"


def _install_profile_shim():
    """antenv.axon_hooks shim so trace=True captures NTFF under axon."""
    if "antenv.axon_hooks" in sys.modules:
        return
    so_path = "/opt/axon/libaxon_pjrt.so"
    try:
        lib = ctypes.CDLL(so_path)
    except OSError:
        return
    if not hasattr(lib, "axon_start_nrt_profile"):
        return
    lib.axon_start_nrt_profile.argtypes = [ctypes.POINTER(ctypes.c_int64), ctypes.c_size_t]
    lib.axon_start_nrt_profile.restype = ctypes.c_int64
    lib.axon_stop_nrt_profile.argtypes = [ctypes.c_char_p]
    lib.axon_stop_nrt_profile.restype = ctypes.c_int64

    @contextlib.contextmanager
    def _hook(output_dir, device_ids):
        import jax

        jax.devices()
        if device_ids:
            ids = (ctypes.c_int64 * len(device_ids))(*device_ids)
            rc = lib.axon_start_nrt_profile(ids, len(device_ids))
        else:
            rc = lib.axon_start_nrt_profile(None, 0)
        if rc != 0:
            raise RuntimeError(f"axon_start_nrt_profile rc={rc}")
        try:
            yield
        finally:
            n = lib.axon_stop_nrt_profile(str(output_dir).encode())
            if n < 0:
                raise RuntimeError(f"axon_stop_nrt_profile rc={n}")

    mod = types.ModuleType("antenv.axon_hooks")
    state = {"hook": _hook}
    mod.set_axon_ntff_profile_hook = lambda h: state.update(hook=h)
    mod.get_axon_ntff_profile_hook = lambda: state["hook"]
    sys.modules["antenv.axon_hooks"] = mod
    try:
        import antenv

        antenv.axon_hooks = mod
    except ImportError:
        pass


def build():
    nc = bacc.Bacc("TRN2", target_bir_lowering=False, debug=False, num_devices=NCORES)

    xt_ext = nc.dram_tensor("xt", [HID, S], BF16, kind="ExternalInput")
    wq_ext = nc.dram_tensor("wq", [HID, HQ * D], BF16, kind="ExternalInput")
    wk_ext = nc.dram_tensor("wk", [HID, D], BF16, kind="ExternalInput")
    wv_ext = nc.dram_tensor("wv", [HID, D], BF16, kind="ExternalInput")
    wo_ext = nc.dram_tensor("wo", [HQ * D, HID], BF16, kind="ExternalInput")
    gq_ext = nc.dram_tensor("gq", [D, 1], BF16, kind="ExternalInput")
    gk_ext = nc.dram_tensor("gk", [D, 1], BF16, kind="ExternalInput")
    tri_ext = nc.dram_tensor("tri", [128, 128], BF16, kind="ExternalInput")
    ones_ext = nc.dram_tensor("ones", [128, 1], BF16, kind="ExternalInput")
    out_ext = nc.dram_tensor("out", [S, HID], BF16, kind="ExternalOutput")

    Exp = mybir.ActivationFunctionType.Exp
    Ln = mybir.ActivationFunctionType.Ln
    Copy = mybir.ActivationFunctionType.Copy
    mult = mybir.AluOpType.mult
    scale_qk = float(D) ** -0.5

    with tile.TileContext(nc) as tc, contextlib.ExitStack() as ctx, \
            nc.allow_low_precision("bf16 softmax accumulators; tolerance 2e-2"):
        wpool = ctx.enter_context(tc.tile_pool(name="w", bufs=1))
        cpool = ctx.enter_context(tc.tile_pool(name="c", bufs=1))
        xtp = ctx.enter_context(tc.tile_pool(name="xt", bufs=36))
        kvp = ctx.enter_context(tc.tile_pool(name="kv", bufs=1))
        qtp = ctx.enter_context(tc.tile_pool(name="qt", bufs=9))
        qsbp = ctx.enter_context(tc.tile_pool(name="qsb", bufs=7))
        sqp = ctx.enter_context(tc.tile_pool(name="sq", bufs=2))
        ptp = ctx.enter_context(tc.tile_pool(name="pt", bufs=6))
        accp = ctx.enter_context(tc.tile_pool(name="accp", bufs=2))
        otp = ctx.enter_context(tc.tile_pool(name="ot", bufs=9))
        rowp = ctx.enter_context(tc.tile_pool(name="rows", bufs=14))
        bcp = ctx.enter_context(tc.tile_pool(name="bc", bufs=6))
        outp = ctx.enter_context(tc.tile_pool(name="outev", bufs=4))
        # PSUM: bigps 2x2 (raws + ST pairs) + rows 1 + ot 2 + op 1 = 8 banks
        bigps = ctx.enter_context(tc.tile_pool(name="bigps", bufs=2, space="PSUM"))
        rowps = ctx.enter_context(tc.tile_pool(name="rowps", bufs=1, space="PSUM"))
        otps = ctx.enter_context(tc.tile_pool(name="otps", bufs=2, space="PSUM"))
        opps = ctx.enter_context(tc.tile_pool(name="opps", bufs=1, space="PSUM"))

        # ---- weights + strip-0 xT, interleaved per k so the first Q chain
        # can start as soon as its operands land.
        xt_tiles = {}
        wq_t, wk_t, wv_t = [], [], []
        for k in range(KT):
            wq_k = wpool.tile([128, HQ * D], BF16, name=f"wq{k}", tag=f"wq{k}")
            nc.sync.dma_start(out=wq_k[:], in_=wq_ext[k * 128:(k + 1) * 128, :])
            wk_k = wpool.tile([128, D], BF16, name=f"wk{k}", tag=f"wk{k}")
            nc.sync.dma_start(out=wk_k[:], in_=wk_ext[k * 128:(k + 1) * 128, :])
            wv_k = wpool.tile([128, D], BF16, name=f"wv{k}", tag=f"wv{k}")
            nc.sync.dma_start(out=wv_k[:], in_=wv_ext[k * 128:(k + 1) * 128, :])
            wq_t.append(wq_k)
            wk_t.append(wk_k)
            wv_t.append(wv_k)
            xt_k = xtp.tile([128, STRIP], BF16, name=f"xt0_{k}", tag="xt")
            nc.sync.dma_start(out=xt_k[:], in_=xt_ext[k * 128:(k + 1) * 128, 0:STRIP])
            xt_tiles[(0, k)] = xt_k
        gq_sb = cpool.tile([D, 1], BF16, name="gq_sb", tag="gq_sb")
        nc.sync.dma_start(out=gq_sb[:], in_=gq_ext[:])
        gk_sb = cpool.tile([D, 1], BF16, name="gk_sb", tag="gk_sb")
        nc.sync.dma_start(out=gk_sb[:], in_=gk_ext[:])
        tri_sb = cpool.tile([128, 128], BF16, name="tri_sb", tag="tri_sb")
        nc.sync.dma_start(out=tri_sb[:], in_=tri_ext[:])
        ones_sb = cpool.tile([128, 1], BF16, name="ones_sb", tag="ones_sb")
        nc.sync.dma_start(out=ones_sb[:], in_=ones_ext[:])
        wo_t = []
        for h in range(HQ):
            wo_h = wpool.tile([128, HID], BF16, name=f"wo{h}", tag=f"wo{h}")
            nc.sync.dma_start(out=wo_h[:], in_=wo_ext[h * 128:(h + 1) * 128, :])
            wo_t.append(wo_h)
        eps_sb = cpool.tile([1, 1], F32, name="eps_sb", tag="eps_sb")
        nc.vector.memset(eps_sb[:], EPS)

        kt_strips = []   # K-hat-T strips [128 d, STRIP tok] bf16, persistent
        v_strips = []    # V strips [128 tok, 4*128 d] bf16 (col block tc = chunk)
        pending = []     # out-proj chunk closures for the previous strip

        def prefetch_xt(s):
            if s >= NSTRIP or (s, 0) in xt_tiles:
                return
            for k in range(KT):
                xt_k = xtp.tile([128, STRIP], BF16, name=f"xt{s}_{k}", tag="xt")
                nc.sync.dma_start(
                    out=xt_k[:],
                    in_=xt_ext[k * 128:(k + 1) * 128, s * STRIP:(s + 1) * STRIP])
                xt_tiles[(s, k)] = xt_k

        def make_op_chunks(sp, ot_heads, evict_on_dve):
            """16 closures; each emits one [128 tok, 512 hid] out-proj tile of
            strip sp: 4 accumulating matmuls + evict + DMA."""
            chunks = []
            for tb in range(4):
                for hs in range(4):
                    def emit(tb=tb, hs=hs):
                        tok0 = sp * STRIP + tb * 128
                        op_ps = opps.tile([128, STRIP], F32,
                                          name=f"op{sp}_{tb}_{hs}", tag="opps")
                        for h in range(HQ):
                            nc.tensor.matmul(
                                op_ps[:],
                                ot_heads[h][:, tb * 128:(tb + 1) * 128],
                                wo_t[h][:, hs * STRIP:(hs + 1) * STRIP],
                                start=(h == 0), stop=(h == HQ - 1),
                            )
                        ob = outp.tile([128, STRIP], BF16,
                                       name=f"ob{sp}_{tb}_{hs}", tag="ob")
                        if evict_on_dve:
                            nc.vector.tensor_copy(ob[:], op_ps[:])
                        else:
                            nc.scalar.activation(ob[:], op_ps[:], Copy)
                        nc.sync.dma_start(
                            out=out_ext[tok0:tok0 + 128, hs * STRIP:(hs + 1) * STRIP],
                            in_=ob[:],
                        )
                    chunks.append(emit)
            return chunks

        def proj(s):
            """Q/K/V projections + RMS norm for strip s. Scalar engine only
            runs Copy/Square-free ops here: Copy evictions + one Ln + one Exp."""
            xt = [xt_tiles[(s, k)] for k in range(KT)]
            prefetch_xt(s + 1)

            raws, lnrs, rms_rows = [], [], []
            pend_ss = []  # ss row matmuls deferred one chain (PE continuity)

            def emit_ss(i):
                # 1/rms = exp(-0.5*ln(ss/D + eps)): avoids Sqrt (wrong table)
                ss = rowps.tile([1, STRIP], F32, name=f"ss{s}_{i}", tag="rowps")
                nc.tensor.matmul(ss[:], ones_sb[:], sqs[i][:],
                                 start=True, stop=True)
                lnr = rowp.tile([1, STRIP], F32, name=f"lnr{s}_{i}", tag="rows")
                nc.scalar.activation(lnr[:], ss[:], Ln, bias=eps_sb[:],
                                     scale=1.0 / D)
                lnrs.append(lnr)

            sqs = []
            # 4 q heads + k: identical handling (k is slot 4).
            # Act-stream batching: [Copy,Ln]x5 first, then Exp x5 below, so
            # the activation-table switches cost 2 loads/strip, not 2/head.
            for i in range(5):
                raw = bigps.tile([128, STRIP], F32, name=f"raw{s}_{i}", tag="bigps")
                for k in range(KT):
                    lhs = wq_t[k][:, i * D:(i + 1) * D] if i < HQ else wk_t[k][:]
                    nc.tensor.matmul(raw[:], lhs, xt[k][:],
                                     start=(k == 0), stop=(k == KT - 1))
                if pend_ss:
                    emit_ss(pend_ss.pop(0))
                sb = qsbp.tile([128, STRIP], BF16, name=f"sb{s}_{i}", tag="qsb")
                nc.scalar.activation(sb[:], raw[:], Copy)
                sq = sqp.tile([128, STRIP], BF16, name=f"sq{s}_{i}", tag="sq")
                nc.vector.tensor_mul(sq[:], sb[:], sb[:])
                sqs.append(sq)
                pend_ss.append(i)
                raws.append(sb)
            # V in [tok, d] layout directly: lhsT = xt chunk, rhs = wv
            vt_ps = bigps.tile([128, STRIP], F32, name=f"vt{s}", tag="bigps")
            for tc in range(4):
                if tc == 2 and pend_ss:
                    emit_ss(pend_ss.pop(0))
                for k in range(KT):
                    nc.tensor.matmul(
                        vt_ps[:, tc * 128:(tc + 1) * 128],
                        xt[k][:, tc * 128:(tc + 1) * 128], wv_t[k][:],
                        start=(k == 0), stop=(k == KT - 1))
            while pend_ss:
                emit_ss(pend_ss.pop(0))
            v_sb = kvp.tile([128, STRIP], BF16, name=f"v{s}", tag="v", bufs=NSTRIP)
            nc.vector.tensor_copy(v_sb[:], vt_ps[:])
            v_strips.append(v_sb)
            for i in range(5):
                rms = rowp.tile([1, STRIP], BF16, name=f"rms{s}_{i}", tag="rows")
                nc.scalar.activation(rms[:], lnrs[i][:], Exp, scale=-0.5)
                rms_rows.append(rms)
            qt_h = []
            for i in range(5):
                bc = bcp.tile([128, STRIP], BF16, name=f"bc{s}_{i}", tag="bc")
                nc.gpsimd.partition_broadcast(bc[:], rms_rows[i][:])
                g = gq_sb if i < HQ else gk_sb
                if i < HQ:
                    qn = qtp.tile([128, STRIP], BF16, name=f"qt{s}_{i}", tag="qt")
                    qt_h.append(qn)
                else:
                    qn = kvp.tile([128, STRIP], BF16, name=f"kt{s}", tag="kt",
                                  bufs=NSTRIP)
                    kt_strips.append(qn)
                nc.vector.scalar_tensor_tensor(qn[:], raws[i][:], g[:], bc[:],
                                               mult, mult)
            return qt_h

        qt_by_strip = {}

        def attn(s):
            """Causal attention for q-strip s over k-tiles 0..4s+3, with
            pending out-proj chunks of strip s-1 interleaved into the PE
            stream to keep it busy while the scalar engine runs Exp."""
            qt_h = qt_by_strip[s]
            nkt = 4 * s + 4
            total_slots = HQ * (nkt // 2)
            n_chunks = len(pending)
            state = {"slot": 0, "emitted": 0}

            def tick():
                state["slot"] += 1
                want = (state["slot"] * n_chunks) // total_slots
                while state["emitted"] < want and pending:
                    pending.pop(0)()
                    state["emitted"] += 1

            ot_heads = []
            fin = {"f": None}  # deferred den/normalize chain of the prev head

            def make_fin(acc, ot_sb, h):
                def f():
                    den = rowps.tile([1, STRIP], F32, name=f"den{s}_{h}",
                                     tag="rowps")
                    nc.tensor.matmul(den[:], ones_sb[:], acc[:],
                                     start=True, stop=True)
                    dst = rowp.tile([1, STRIP], F32, name=f"dst{s}_{h}",
                                    tag="rows")
                    nc.vector.tensor_copy(dst[:], den[:])
                    rd = rowp.tile([1, STRIP], F32, name=f"rd{s}_{h}",
                                   tag="rows")
                    nc.vector.reciprocal_approx_fast(rd[:], dst[:])
                    bcd = bcp.tile([128, STRIP], F32, name=f"bcd{s}_{h}",
                                   tag="bc")
                    nc.gpsimd.partition_broadcast(bcd[:], rd[:])
                    nc.vector.tensor_tensor(ot_sb[:], ot_sb[:], bcd[:], mult)
                return f

            for h in range(HQ):
                ot_ps = otps.tile([128, STRIP], F32, name=f"ot{s}_{h}", tag="otps")
                acc = accp.tile([128, STRIP], BF16, name=f"acc{s}_{h}", tag="acc")
                npair = nkt // 2
                pts = [None] * npair

                def issue_st_pair(p, h=h, pts=pts):
                    # two 512-col ST matmuls into one 2-bank [128,1024] PSUM
                    # tile; ONE Exp covers both (halves the Act per-op cost)
                    st2 = bigps.tile([128, 2 * STRIP], F32,
                                     name=f"st{s}_{h}_{p}", tag="bigps")
                    for half in range(2):
                        k = 2 * p + half
                        nc.tensor.matmul(
                            st2[:, half * STRIP:(half + 1) * STRIP],
                            kt_strips[k // 4][:, (k % 4) * 128:(k % 4 + 1) * 128],
                            qt_h[h][:],
                            start=True, stop=True,
                        )
                    pt2 = ptp.tile([128, 2 * STRIP], BF16,
                                   name=f"pt{s}_{h}_{p}", tag="pt")
                    nc.scalar.activation(pt2[:], st2[:], Exp, scale=scale_qk)
                    for half in range(2):
                        k = 2 * p + half
                        jj = k - 4 * s
                        if jj >= 0:
                            off = half * STRIP
                            c0 = 128 * jj
                            if c0 > 0:
                                nc.gpsimd.memset(pt2[:, off:off + c0], 0.0)
                            nc.vector.tensor_tensor(
                                pt2[:, off + c0:off + c0 + 128],
                                pt2[:, off + c0:off + c0 + 128],
                                tri_sb[:], mult)
                    pts[p] = pt2

                def issue_pv_pair(p, ot_ps=ot_ps, acc=acc, pts=pts, nkt=nkt):
                    pt2 = pts[p]
                    for half in range(2):
                        k = 2 * p + half
                        jj = k - 4 * s
                        m0 = 0 if (jj <= 0 or k == 0) else 128 * jj
                        off = half * STRIP
                        nc.tensor.matmul(
                            ot_ps[:, m0:],
                            v_strips[k // 4][:, (k % 4) * 128:(k % 4 + 1) * 128],
                            pt2[:, off + m0:off + STRIP],
                            start=(k == 0), stop=(k == nkt - 1),
                        )
                        if k == 0:
                            nc.vector.tensor_copy(acc[:], pt2[:, 0:STRIP])
                        else:
                            nc.vector.tensor_add(acc[:], acc[:],
                                                 pt2[:, off:off + STRIP])

                issue_st_pair(0)
                for p in range(1, npair):
                    issue_st_pair(p)
                    issue_pv_pair(p - 1)
                    if p == 1 and fin["f"] is not None:
                        fin["f"]()  # prev head's den matmul lands here so the
                        fin["f"] = None  # PE never waits on the DVE acc chain
                    tick()
                issue_pv_pair(npair - 1)
                if fin["f"] is not None:
                    fin["f"]()  # s==0 heads have a single pair
                    fin["f"] = None
                tick()

                # evict OT unnormalized immediately; den/normalize deferred
                ot_sb = otp.tile([128, STRIP], BF16, name=f"otsb{s}_{h}", tag="ot")
                nc.vector.tensor_copy(ot_sb[:], ot_ps[:])
                fin["f"] = make_fin(acc, ot_sb, h)
                ot_heads.append(ot_sb)
            fin["f"]()
            while pending:
                pending.pop(0)()
            return ot_heads

        # strip-level software pipeline: proj runs one strip ahead of attn
        qt_by_strip[0] = proj(0)
        for s in range(NSTRIP):
            if s + 1 < NSTRIP:
                qt_by_strip[s + 1] = proj(s + 1)
            ot_heads = attn(s)
            pending.extend(make_op_chunks(s, ot_heads, evict_on_dve=(s >= 2)))
        while pending:
            pending.pop(0)()

    nc.compile()
    return nc


_NC_CACHE = None
last_result = None


def _tri_np():
    kr = np.arange(128)[:, None]
    qc = np.arange(128)[None, :]
    return np.where(kr <= qc, 1.0, 0.0).astype(ml_dtypes.bfloat16)


def kernel(x, Wq, Wk, Wv, Wo, gq, gk):
    global _NC_CACHE, last_result
    bf = ml_dtypes.bfloat16
    x = np.asarray(x, np.float32)
    Wq = np.asarray(Wq, np.float32).astype(bf)
    Wk = np.asarray(Wk, np.float32).astype(bf)
    Wv = np.asarray(Wv, np.float32).astype(bf)
    Wo = np.asarray(Wo, np.float32).astype(bf)
    gq = np.asarray(gq, np.float32)
    gk = np.asarray(gk, np.float32)

    tri = _tri_np()
    ones = np.ones((128, 1), bf)
    in_maps = []
    for core in range(NCORES):
        b, g = core // 4, core % 4
        in_maps.append({
            "xt": np.ascontiguousarray(x[b].T).astype(bf),
            "wq": np.ascontiguousarray(Wq[:, g * HQ * D:(g + 1) * HQ * D]),
            "wk": np.ascontiguousarray(Wk[:, g * D:(g + 1) * D]),
            "wv": np.ascontiguousarray(Wv[:, g * D:(g + 1) * D]),
            "wo": np.ascontiguousarray(Wo[g * HQ * D:(g + 1) * HQ * D, :]),
            "gq": np.ascontiguousarray(gq.reshape(D, 1)).astype(bf),
            "gk": np.ascontiguousarray(gk.reshape(D, 1)).astype(bf),
            "tri": tri,
            "ones": ones,
        })

    if TRACE:
        _install_profile_shim()
    if _NC_CACHE is None:
        _NC_CACHE = build()
    last_result = run_bass_kernel_spmd(
        _NC_CACHE, in_maps, core_ids=list(range(NCORES)), trace=TRACE
    )
    out = np.zeros((2, S, HID), np.float32)
    for core in range(NCORES):
        out[core // 4] += last_result.results[core]["out"].astype(np.float32)
    return out


# revision 15
# speedup vs baseline: 1.0079x; 1.0079x over previous
"""Distributed Bass kernel for nn_Attention_75514114998541.

GQA attention block (16 Q heads / 4 KV heads, head_dim 128, hidden 2048,
B=2, S=2048) with per-head RMSNorm on q/k, causal softmax, output proj.

Sharding: 8 cores = 2 (batch) x 4 (head groups). Core 4*b+g handles batch b
and heads [4g, 4g+4) (= kv head g). Wq/Wk/Wv column-sharded, Wo row-sharded;
each core emits a partial [S, HID] output (bf16), host sums the 4 partials
per batch in fp32.

v2 design (vs fp32r baseline):
  * all SBUF activations/weights in bf16 (PSUM accumulation stays fp32):
    halves DMA, enables DVE 2x/4x perf modes. PE rate is the same as fp32r
    but per-instruction overheads shrink and diagonal matmuls can shrink
    below 256 columns.
  * single activation table: scalar engine only ever runs Copy / Square /
    Ln / Exp (all in natural_log_exp_and_others) -- the RMS-norm rsqrt is
    computed as exp(-0.5*ln(ms+eps)), so no 1283ns table reloads.
  * V is produced directly in [tok, d] layout (64 N=128 matmuls per strip
    instead of 16 N=512 + 4 PE transposes).
  * rms rows for 4 q heads + k are accumulated into one [5,512] PSUM tile
    (matmul writes at partition offset h) -> one Ln + one Exp for all 5.
  * softmax accumulator (denominator) adds run on DVE in bf16.
  * out-proj of strip s-1 is interleaved INSIDE strip s's attention k-loop
    (1 chunk per few tiles) to keep the PE p-state ramped; projections of
    strip s+1 are emitted BEFORE attention of strip s so the PE never
    waits on the scalar engine at strip boundaries.
Layouts: xT[hid, tok] (host pre-transpose) -> QT/KT[d, tok] -> ST[k, q]
  -> PT[k, q] -> OT[d, q] -> out[tok, hid].
"""
import contextlib
import ctypes
import os
import sys
import types

import numpy as np
import ml_dtypes

sys.path.insert(0, "/opt/trn_rl_repo")

import concourse.bacc as bacc
import concourse.mybir as mybir
import concourse.tile as tile
from concourse.bass_utils import run_bass_kernel_spmd

F32 = mybir.dt.float32
BF16 = mybir.dt.bfloat16

NCORES = 8
S = 2048            # sequence length (= tokens per batch)
HID = 2048          # hidden dim
D = 128             # head dim
HQ = 4              # q heads per core
STRIP = 512         # token strip (matmul moving free dim)
NSTRIP = S // STRIP          # 4
KT = HID // 128              # 16 hidden k-tiles
EPS = 1e-6
TRACE = os.environ.get("BASS_KERNEL_TRACE", "0") == "1"


def _install_profile_shim():
    """antenv.axon_hooks shim so trace=True captures NTFF under axon."""
    if "antenv.axon_hooks" in sys.modules:
        return
    so_path = "/opt/axon/libaxon_pjrt.so"
    try:
        lib = ctypes.CDLL(so_path)
    except OSError:
        return
    if not hasattr(lib, "axon_start_nrt_profile"):
        return
    lib.axon_start_nrt_profile.argtypes = [ctypes.POINTER(ctypes.c_int64), ctypes.c_size_t]
    lib.axon_start_nrt_profile.restype = ctypes.c_int64
    lib.axon_stop_nrt_profile.argtypes = [ctypes.c_char_p]
    lib.axon_stop_nrt_profile.restype = ctypes.c_int64

    @contextlib.contextmanager
    def _hook(output_dir, device_ids):
        import jax

        jax.devices()
        if device_ids:
            ids = (ctypes.c_int64 * len(device_ids))(*device_ids)
            rc = lib.axon_start_nrt_profile(ids, len(device_ids))
        else:
            rc = lib.axon_start_nrt_profile(None, 0)
        if rc != 0:
            raise RuntimeError(f"axon_start_nrt_profile rc={rc}")
        try:
            yield
        finally:
            n = lib.axon_stop_nrt_profile(str(output_dir).encode())
            if n < 0:
                raise RuntimeError(f"axon_stop_nrt_profile rc={n}")

    mod = types.ModuleType("antenv.axon_hooks")
    state = {"hook": _hook}
    mod.set_axon_ntff_profile_hook = lambda h: state.update(hook=h)
    mod.get_axon_ntff_profile_hook = lambda: state["hook"]
    sys.modules["antenv.axon_hooks"] = mod
    try:
        import antenv

        antenv.axon_hooks = mod
    except ImportError:
        pass


def build():
    nc = bacc.Bacc("TRN2", target_bir_lowering=False, debug=False, num_devices=NCORES)

    xt_ext = nc.dram_tensor("xt", [HID, S], BF16, kind="ExternalInput")
    wq_ext = nc.dram_tensor("wq", [HID, HQ * D], BF16, kind="ExternalInput")
    wk_ext = nc.dram_tensor("wk", [HID, D], BF16, kind="ExternalInput")
    wv_ext = nc.dram_tensor("wv", [HID, D], BF16, kind="ExternalInput")
    wo_ext = nc.dram_tensor("wo", [HQ * D, HID], BF16, kind="ExternalInput")
    gq_ext = nc.dram_tensor("gq", [D, 1], BF16, kind="ExternalInput")
    gk_ext = nc.dram_tensor("gk", [D, 1], BF16, kind="ExternalInput")
    tri_ext = nc.dram_tensor("tri", [128, 128], BF16, kind="ExternalInput")
    ones_ext = nc.dram_tensor("ones", [128, 1], BF16, kind="ExternalInput")
    out_ext = nc.dram_tensor("out", [S, HID], BF16, kind="ExternalOutput")

    Exp = mybir.ActivationFunctionType.Exp
    Ln = mybir.ActivationFunctionType.Ln
    Copy = mybir.ActivationFunctionType.Copy
    mult = mybir.AluOpType.mult
    scale_qk = float(D) ** -0.5

    with tile.TileContext(nc) as tc, contextlib.ExitStack() as ctx, \
            nc.allow_low_precision("bf16 softmax accumulators; tolerance 2e-2"):
        wpool = ctx.enter_context(tc.tile_pool(name="w", bufs=1))
        cpool = ctx.enter_context(tc.tile_pool(name="c", bufs=1))
        xtp = ctx.enter_context(tc.tile_pool(name="xt", bufs=36))
        kvp = ctx.enter_context(tc.tile_pool(name="kv", bufs=1))
        qtp = ctx.enter_context(tc.tile_pool(name="qt", bufs=9))
        qsbp = ctx.enter_context(tc.tile_pool(name="qsb", bufs=7))
        sqp = ctx.enter_context(tc.tile_pool(name="sq", bufs=2))
        ptp = ctx.enter_context(tc.tile_pool(name="pt", bufs=6))
        accp = ctx.enter_context(tc.tile_pool(name="accp", bufs=2))
        otp = ctx.enter_context(tc.tile_pool(name="ot", bufs=9))
        rowp = ctx.enter_context(tc.tile_pool(name="rows", bufs=14))
        bcp = ctx.enter_context(tc.tile_pool(name="bc", bufs=6))
        outp = ctx.enter_context(tc.tile_pool(name="outev", bufs=4))
        # PSUM: bigps 2x2 (raws + ST pairs) + rows 1 + ot 2 + op 1 = 8 banks
        bigps = ctx.enter_context(tc.tile_pool(name="bigps", bufs=2, space="PSUM"))
        rowps = ctx.enter_context(tc.tile_pool(name="rowps", bufs=1, space="PSUM"))
        otps = ctx.enter_context(tc.tile_pool(name="otps", bufs=1, space="PSUM"))
        opps = ctx.enter_context(tc.tile_pool(name="opps", bufs=2, space="PSUM"))

        # ---- weights + strip-0 xT, interleaved per k so the first Q chain
        # can start as soon as its operands land.
        xt_tiles = {}
        wq_t, wk_t, wv_t = [], [], []
        # DMA order matched to first use: q0's operands first so the PE can
        # start ~8us in, remaining head columns / wk / wv behind them.
        for k in range(KT):
            xt_k = xtp.tile([128, STRIP], BF16, name=f"xt0_{k}", tag="xt")
            nc.sync.dma_start(out=xt_k[:], in_=xt_ext[k * 128:(k + 1) * 128, 0:STRIP])
            xt_tiles[(0, k)] = xt_k
            wq_k = wpool.tile([128, HQ * D], BF16, name=f"wq{k}", tag=f"wq{k}")
            nc.sync.dma_start(out=wq_k[:, 0:D],
                              in_=wq_ext[k * 128:(k + 1) * 128, 0:D])
            wq_t.append(wq_k)
        for h in range(1, HQ):
            for k in range(KT):
                nc.sync.dma_start(
                    out=wq_t[k][:, h * D:(h + 1) * D],
                    in_=wq_ext[k * 128:(k + 1) * 128, h * D:(h + 1) * D])
        for k in range(KT):
            wk_k = wpool.tile([128, D], BF16, name=f"wk{k}", tag=f"wk{k}")
            nc.sync.dma_start(out=wk_k[:], in_=wk_ext[k * 128:(k + 1) * 128, :])
            wv_k = wpool.tile([128, D], BF16, name=f"wv{k}", tag=f"wv{k}")
            nc.sync.dma_start(out=wv_k[:], in_=wv_ext[k * 128:(k + 1) * 128, :])
            wk_t.append(wk_k)
            wv_t.append(wv_k)
        gq_sb = cpool.tile([D, 1], BF16, name="gq_sb", tag="gq_sb")
        nc.sync.dma_start(out=gq_sb[:], in_=gq_ext[:])
        gk_sb = cpool.tile([D, 1], BF16, name="gk_sb", tag="gk_sb")
        nc.sync.dma_start(out=gk_sb[:], in_=gk_ext[:])
        tri_sb = cpool.tile([128, 128], BF16, name="tri_sb", tag="tri_sb")
        nc.sync.dma_start(out=tri_sb[:], in_=tri_ext[:])
        ones_sb = cpool.tile([128, 1], BF16, name="ones_sb", tag="ones_sb")
        nc.sync.dma_start(out=ones_sb[:], in_=ones_ext[:])
        wo_t = []

        def load_wo():
            # deferred until after proj(1) so xt prefetches win the DMA queue
            for h in range(HQ):
                wo_h = wpool.tile([128, HID], BF16, name=f"wo{h}", tag=f"wo{h}")
                nc.sync.dma_start(out=wo_h[:], in_=wo_ext[h * 128:(h + 1) * 128, :])
                wo_t.append(wo_h)
        eps_sb = cpool.tile([1, 1], F32, name="eps_sb", tag="eps_sb")
        nc.vector.memset(eps_sb[:], EPS)

        kt_strips = []   # K-hat-T strips [128 d, STRIP tok] bf16, persistent
        v_strips = []    # V strips [128 tok, 4*128 d] bf16 (col block tc = chunk)
        pending = []     # out-proj chunk closures for the previous strip

        def prefetch_xt(s):
            if s >= NSTRIP or (s, 0) in xt_tiles:
                return
            for k in range(KT):
                xt_k = xtp.tile([128, STRIP], BF16, name=f"xt{s}_{k}", tag="xt")
                nc.sync.dma_start(
                    out=xt_k[:],
                    in_=xt_ext[k * 128:(k + 1) * 128, s * STRIP:(s + 1) * STRIP])
                xt_tiles[(s, k)] = xt_k

        def make_op_chunks(sp, ot_heads, tail=False):
            """16 closures; each emits one [128 tok, 512 hid] out-proj tile of
            strip sp: 4 accumulating matmuls + evict + DMA. Tail chunks also
            rotate through the (then-idle) otps banks to overlap evictions."""
            chunks = []
            for tb in range(4):
                for hs in range(4):
                    def emit(tb=tb, hs=hs):
                        use_ot = tail and (tb + hs) % 2
                        pool = otps if use_ot else opps
                        tok0 = sp * STRIP + tb * 128
                        op_ps = pool.tile([128, STRIP], F32,
                                          name=f"op{sp}_{tb}_{hs}",
                                          tag="otps" if use_ot else "opps")
                        for h in range(HQ):
                            nc.tensor.matmul(
                                op_ps[:],
                                ot_heads[h][:, tb * 128:(tb + 1) * 128],
                                wo_t[h][:, hs * STRIP:(hs + 1) * STRIP],
                                start=(h == 0), stop=(h == HQ - 1),
                            )
                        ob = outp.tile([128, STRIP], BF16,
                                       name=f"ob{sp}_{tb}_{hs}", tag="ob")
                        if (tb + hs) % 2:
                            nc.scalar.activation(ob[:], op_ps[:], Copy)
                        else:
                            nc.vector.tensor_copy(ob[:], op_ps[:])
                        nc.sync.dma_start(
                            out=out_ext[tok0:tok0 + 128, hs * STRIP:(hs + 1) * STRIP],
                            in_=ob[:],
                        )
                    chunks.append(emit)
            return chunks

        def proj(s):
            """Q/K/V projections + RMS norm for strip s. Scalar engine only
            runs Copy/Square-free ops here: Copy evictions + one Ln + one Exp."""
            xt = [xt_tiles[(s, k)] for k in range(KT)]
            prefetch_xt(s + 1)

            raws, lnrs, rms_rows = [], [], []
            pend_ss = []  # ss row matmuls deferred one chain (PE continuity)

            def emit_ss(i):
                # 1/rms = exp(-0.5*ln(ss/D + eps)): avoids Sqrt (wrong table)
                ss = rowps.tile([1, STRIP], F32, name=f"ss{s}_{i}", tag="rowps")
                nc.tensor.matmul(ss[:], ones_sb[:], sqs[i][:],
                                 start=True, stop=True)
                lnr = rowp.tile([1, STRIP], F32, name=f"lnr{s}_{i}", tag="rows")
                nc.scalar.activation(lnr[:], ss[:], Ln, bias=eps_sb[:],
                                     scale=1.0 / D)
                lnrs.append(lnr)

            sqs = []
            # 4 q heads + k: identical handling (k is slot 4).
            # Act-stream batching: [Copy,Ln]x5 first, then Exp x5 below, so
            # the activation-table switches cost 2 loads/strip, not 2/head.
            for i in range(5):
                raw = bigps.tile([128, STRIP], F32, name=f"raw{s}_{i}", tag="bigps")
                for k in range(KT):
                    lhs = wq_t[k][:, i * D:(i + 1) * D] if i < HQ else wk_t[k][:]
                    nc.tensor.matmul(raw[:], lhs, xt[k][:],
                                     start=(k == 0), stop=(k == KT - 1))
                if pend_ss:
                    emit_ss(pend_ss.pop(0))
                sb = qsbp.tile([128, STRIP], BF16, name=f"sb{s}_{i}", tag="qsb")
                nc.scalar.activation(sb[:], raw[:], Copy)
                sq = sqp.tile([128, STRIP], BF16, name=f"sq{s}_{i}", tag="sq")
                nc.vector.tensor_mul(sq[:], sb[:], sb[:])
                sqs.append(sq)
                pend_ss.append(i)
                raws.append(sb)
            # V in [tok, d] layout directly: lhsT = xt chunk, rhs = wv
            vt_ps = bigps.tile([128, STRIP], F32, name=f"vt{s}", tag="bigps")
            for tc in range(4):
                if tc == 2 and pend_ss:
                    emit_ss(pend_ss.pop(0))
                for k in range(KT):
                    nc.tensor.matmul(
                        vt_ps[:, tc * 128:(tc + 1) * 128],
                        xt[k][:, tc * 128:(tc + 1) * 128], wv_t[k][:],
                        start=(k == 0), stop=(k == KT - 1))
            while pend_ss:
                emit_ss(pend_ss.pop(0))
            v_sb = kvp.tile([128, STRIP], BF16, name=f"v{s}", tag="v", bufs=NSTRIP)
            nc.vector.tensor_copy(v_sb[:], vt_ps[:])
            v_strips.append(v_sb)
            for i in range(5):
                rms = rowp.tile([1, STRIP], BF16, name=f"rms{s}_{i}", tag="rows")
                nc.scalar.activation(rms[:], lnrs[i][:], Exp, scale=-0.5)
                rms_rows.append(rms)
            qt_h = []
            for i in range(5):
                bc = bcp.tile([128, STRIP], BF16, name=f"bc{s}_{i}", tag="bc")
                nc.gpsimd.partition_broadcast(bc[:], rms_rows[i][:])
                g = gq_sb if i < HQ else gk_sb
                if i < HQ:
                    qn = qtp.tile([128, STRIP], BF16, name=f"qt{s}_{i}", tag="qt")
                    qt_h.append(qn)
                else:
                    qn = kvp.tile([128, STRIP], BF16, name=f"kt{s}", tag="kt",
                                  bufs=NSTRIP)
                    kt_strips.append(qn)
                nc.vector.scalar_tensor_tensor(qn[:], raws[i][:], g[:], bc[:],
                                               mult, mult)
            return qt_h

        qt_by_strip = {}

        def attn(s):
            """Causal attention for q-strip s over k-tiles 0..4s+3, with
            pending out-proj chunks of strip s-1 interleaved into the PE
            stream to keep it busy while the scalar engine runs Exp."""
            qt_h = qt_by_strip[s]
            nkt = 4 * s + 4
            total_slots = HQ * (nkt // 2)
            n_chunks = len(pending)
            state = {"slot": 0, "emitted": 0}

            def tick():
                state["slot"] += 1
                want = (state["slot"] * n_chunks) // total_slots
                while state["emitted"] < want and pending:
                    pending.pop(0)()
                    state["emitted"] += 1

            ot_heads = []
            fin = {"f": None}  # deferred den/normalize chain of the prev head

            def make_fin(acc, ot_sb, h):
                def f():
                    den = rowps.tile([1, STRIP], F32, name=f"den{s}_{h}",
                                     tag="rowps")
                    nc.tensor.matmul(den[:], ones_sb[:], acc[:],
                                     start=True, stop=True)
                    dst = rowp.tile([1, STRIP], F32, name=f"dst{s}_{h}",
                                    tag="rows")
                    nc.vector.tensor_copy(dst[:], den[:])
                    rd = rowp.tile([1, STRIP], F32, name=f"rd{s}_{h}",
                                   tag="rows")
                    nc.vector.reciprocal_approx_fast(rd[:], dst[:])
                    bcd = bcp.tile([128, STRIP], F32, name=f"bcd{s}_{h}",
                                   tag="bc")
                    nc.gpsimd.partition_broadcast(bcd[:], rd[:])
                    nc.vector.tensor_tensor(ot_sb[:], ot_sb[:], bcd[:], mult)
                return f

            for h in range(HQ):
                ot_ps = otps.tile([128, STRIP], F32, name=f"ot{s}_{h}", tag="otps")
                acc = accp.tile([128, STRIP], BF16, name=f"acc{s}_{h}", tag="acc")
                npair = nkt // 2
                pts = [None] * npair

                def issue_st_pair(p, h=h, pts=pts):
                    # two 512-col ST matmuls into one 2-bank [128,1024] PSUM
                    # tile; ONE Exp covers both (halves the Act per-op cost)
                    st2 = bigps.tile([128, 2 * STRIP], F32,
                                     name=f"st{s}_{h}_{p}", tag="bigps")
                    for half in range(2):
                        k = 2 * p + half
                        nc.tensor.matmul(
                            st2[:, half * STRIP:(half + 1) * STRIP],
                            kt_strips[k // 4][:, (k % 4) * 128:(k % 4 + 1) * 128],
                            qt_h[h][:],
                            start=True, stop=True,
                        )
                    pt2 = ptp.tile([128, 2 * STRIP], BF16,
                                   name=f"pt{s}_{h}_{p}", tag="pt")
                    nc.scalar.activation(pt2[:], st2[:], Exp, scale=scale_qk)
                    for half in range(2):
                        k = 2 * p + half
                        jj = k - 4 * s
                        if jj >= 0:
                            off = half * STRIP
                            c0 = 128 * jj
                            if c0 > 0:
                                nc.gpsimd.memset(pt2[:, off:off + c0], 0.0)
                            nc.vector.tensor_tensor(
                                pt2[:, off + c0:off + c0 + 128],
                                pt2[:, off + c0:off + c0 + 128],
                                tri_sb[:], mult)
                    pts[p] = pt2

                def issue_pv_pair(p, ot_ps=ot_ps, acc=acc, pts=pts, nkt=nkt):
                    pt2 = pts[p]
                    for half in range(2):
                        k = 2 * p + half
                        jj = k - 4 * s
                        m0 = 0 if (jj <= 0 or k == 0) else 128 * jj
                        off = half * STRIP
                        nc.tensor.matmul(
                            ot_ps[:, m0:],
                            v_strips[k // 4][:, (k % 4) * 128:(k % 4 + 1) * 128],
                            pt2[:, off + m0:off + STRIP],
                            start=(k == 0), stop=(k == nkt - 1),
                        )
                        if k == 0:
                            nc.vector.tensor_copy(acc[:], pt2[:, 0:STRIP])
                        else:
                            nc.vector.tensor_add(acc[:], acc[:],
                                                 pt2[:, off:off + STRIP])

                issue_st_pair(0)
                for p in range(1, npair):
                    issue_st_pair(p)
                    issue_pv_pair(p - 1)
                    if p == 1 and fin["f"] is not None:
                        fin["f"]()  # prev head's den matmul lands here so the
                        fin["f"] = None  # PE never waits on the DVE acc chain
                    tick()
                issue_pv_pair(npair - 1)
                if fin["f"] is not None:
                    fin["f"]()  # s==0 heads have a single pair
                    fin["f"] = None
                tick()

                # evict OT unnormalized immediately; den/normalize deferred
                ot_sb = otp.tile([128, STRIP], BF16, name=f"otsb{s}_{h}", tag="ot")
                nc.vector.tensor_copy(ot_sb[:], ot_ps[:])
                fin["f"] = make_fin(acc, ot_sb, h)
                ot_heads.append(ot_sb)
            fin["f"]()
            while pending:
                pending.pop(0)()
            return ot_heads

        # strip-level software pipeline: proj runs one strip ahead of attn
        qt_by_strip[0] = proj(0)
        for s in range(NSTRIP):
            if s + 1 < NSTRIP:
                qt_by_strip[s + 1] = proj(s + 1)
            if s == 0:
                load_wo()
            ot_heads = attn(s)
            pending.extend(make_op_chunks(s, ot_heads, tail=(s == NSTRIP - 1)))
        while pending:
            pending.pop(0)()

    nc.compile()
    return nc


_NC_CACHE = None
last_result = None


def _tri_np():
    kr = np.arange(128)[:, None]
    qc = np.arange(128)[None, :]
    return np.where(kr <= qc, 1.0, 0.0).astype(ml_dtypes.bfloat16)


def kernel(x, Wq, Wk, Wv, Wo, gq, gk):
    global _NC_CACHE, last_result
    bf = ml_dtypes.bfloat16
    x = np.asarray(x, np.float32)
    Wq = np.asarray(Wq, np.float32).astype(bf)
    Wk = np.asarray(Wk, np.float32).astype(bf)
    Wv = np.asarray(Wv, np.float32).astype(bf)
    Wo = np.asarray(Wo, np.float32).astype(bf)
    gq = np.asarray(gq, np.float32)
    gk = np.asarray(gk, np.float32)

    tri = _tri_np()
    ones = np.ones((128, 1), bf)
    in_maps = []
    for core in range(NCORES):
        b, g = core // 4, core % 4
        in_maps.append({
            "xt": np.ascontiguousarray(x[b].T).astype(bf),
            "wq": np.ascontiguousarray(Wq[:, g * HQ * D:(g + 1) * HQ * D]),
            "wk": np.ascontiguousarray(Wk[:, g * D:(g + 1) * D]),
            "wv": np.ascontiguousarray(Wv[:, g * D:(g + 1) * D]),
            "wo": np.ascontiguousarray(Wo[g * HQ * D:(g + 1) * HQ * D, :]),
            "gq": np.ascontiguousarray(gq.reshape(D, 1)).astype(bf),
            "gk": np.ascontiguousarray(gk.reshape(D, 1)).astype(bf),
            "tri": tri,
            "ones": ones,
        })

    if TRACE:
        _install_profile_shim()
    if _NC_CACHE is None:
        _NC_CACHE = build()
    last_result = run_bass_kernel_spmd(
        _NC_CACHE, in_maps, core_ids=list(range(NCORES)), trace=TRACE
    )
    out = np.zeros((2, S, HID), np.float32)
    for core in range(NCORES):
        out[core // 4] += last_result.results[core]["out"].astype(np.float32)
    return out
